# revision 8
# baseline (speedup 1.0000x reference)
"""Multi-head latent attention on 8 TRN2 NeuronCores (Bass/Tile).

Sharding: core c handles batch b=c//2, query rows half=c%2 (1024 rows).
Keys/values cover the full batch (2048 rows). The host passes x transposed
and "rolled" so each core's query rows are always columns 0-1023, keeping
the SPMD program uniform (softmax is permutation-invariant over keys).

On-chip layouts are transposed ([feature, row]). Scores are computed as
ST[k, q]; softmax denominators come for free from a ones-column appended
to V (M=65 PV matmul). Projections run in float32r (tf32-class precision
at full PE speed); attention tensors are bf16.
"""
import numpy as np
from contextlib import ExitStack

import ml_dtypes

import concourse.bass as bass
import concourse.tile as tile
from concourse import bacc, mybir
from concourse.bass_utils import run_bass_kernel_spmd

P = 128
D = 1024
NKV = 2048
NQ = 1024          # query rows per core
RK = 128           # kv latent rank
H = 16
HD = 64
SCALE = HD ** -0.5  # 0.125
NT = NKV // P      # 16 key-row tiles
DT = D // P        # 8 feature tiles
QC = NQ // 512     # 2 query chunks of 512

F32 = mybir.dt.float32
F32R = mybir.dt.float32r
BF16 = mybir.dt.bfloat16
ADD = mybir.AluOpType.add
MULT = mybir.AluOpType.mult
EXP = mybir.ActivationFunctionType.Exp


def _body(ctx: ExitStack, tc, aps):
    nc = tc.nc
    xq, xkv2, wq, wdown, wkup, wvup, wout, bq, bdown, bk, bv, bout, out = aps

    w1 = ctx.enter_context(tc.tile_pool(name="w1", bufs=1))
    big = ctx.enter_context(tc.tile_pool(name="big", bufs=1))
    ps_a = ctx.enter_context(tc.tile_pool(name="ps_a", bufs=2, space="PSUM"))
    ps_st = ctx.enter_context(tc.tile_pool(name="ps_st", bufs=3, space="PSUM"))
    ps_pv = ctx.enter_context(tc.tile_pool(name="ps_pv", bufs=2, space="PSUM"))
    ps_bc = ctx.enter_context(tc.tile_pool(name="ps_bc", bufs=1, space="PSUM"))

    # ---- weights / biases to SBUF ----
    wq_r = wq.rearrange("(k p) n -> p k n", p=P)
    wdown_sb = w1.tile([P, DT, RK], F32R)
    nc.sync.dma_start(wdown_sb[:], wdown.rearrange("(t p) r -> p t r", p=P))
    wkup_sb = w1.tile([P, D], F32R)
    nc.sync.dma_start(wkup_sb[:], wkup)
    wvup_sb = w1.tile([P, D], F32R)
    nc.sync.dma_start(wvup_sb[:], wvup)
    wout_sb = w1.tile([P, DT, D], BF16)
    nc.sync.dma_start(wout_sb[:], wout.rearrange("(t p) n -> p t n", p=P))

    bq_sb = w1.tile([P, DT], F32)
    nc.sync.dma_start(bq_sb[:], bq.rearrange("(t p) -> p t", p=P))
    bk_sb = w1.tile([P, DT], F32)
    nc.sync.dma_start(bk_sb[:], bk.rearrange("(t p) -> p t", p=P))
    bdown_sb = w1.tile([P, 1], F32)
    nc.sync.dma_start(bdown_sb[:], bdown.rearrange("(o p) -> p o", p=P))
    bv_row = w1.tile([1, D], F32)
    nc.sync.dma_start(bv_row[:], bv.rearrange("(o d) -> o d", o=1))
    bout_row = w1.tile([1, D], F32)
    nc.sync.dma_start(bout_row[:], bout.rearrange("(o d) -> o d", o=1))

    # broadcast bv/bout along partitions via K=1 matmul (fp32)
    ones1 = w1.tile([1, P], F32)
    nc.vector.memset(ones1[:], 1.0)
    bv_bc = w1.tile([P, D], F32)
    bout_bc = w1.tile([P, D], F32)
    for c in range(2):
        pb = ps_bc.tile([P, 512], F32, tag="bc")
        nc.tensor.matmul(pb[:], ones1[:], bv_row[:, c * 512:(c + 1) * 512],
                         start=True, stop=True)
        nc.vector.tensor_copy(bv_bc[:, c * 512:(c + 1) * 512], pb[:])
        pb2 = ps_bc.tile([P, 512], F32, tag="bc")
        nc.tensor.matmul(pb2[:], ones1[:], bout_row[:, c * 512:(c + 1) * 512],
                         start=True, stop=True)
        nc.vector.tensor_copy(bout_bc[:, c * 512:(c + 1) * 512], pb2[:])

    # ones for the denominator-broadcast matmul (row 64 = lane of the denom)
    ones65_f = w1.tile([65, HD], F32)
    nc.vector.memset(ones65_f[:], 1.0)
    ones65 = w1.tile([65, HD], F32R)
    nc.vector.tensor_copy(ones65[:], ones65_f[:])

    # persistent attention tensors
    qT = big.tile([P, DT, NQ], BF16)
    kT = big.tile([P, DT, NKV], BF16)
    v_aug = big.tile([P, NT, H, HD + 1], BF16)
    attnT = big.tile([P, DT, NQ], BF16)
    nc.vector.memset(v_aug[:], 1.0)

    # ---- phase 1: projections (scoped pools, released before attention) --
    with tc.tile_pool(name="ph1", bufs=1) as ph1, \
            tc.tile_pool(name="ph1s", bufs=4) as ph1s:
        xq_sb = ph1.tile([P, DT, NQ], F32R)
        nc.sync.dma_start(xq_sb[:], xq.rearrange("(t p) n -> p t n", p=P))
        xkv2_r = xkv2.rearrange("(t p) n -> p t n", p=P)
        latentT = ph1.tile([P, NKV], F32R)

        # latent projection: latentT[rank, all 2048 rows]
        for c4 in range(4):
            pl = ps_a.tile([P, 512], F32, tag="acc")
            for kt in range(DT):
                if c4 < 2:
                    rhs = xq_sb[:, kt, c4 * 512:(c4 + 1) * 512]
                else:
                    xs = ph1s.tile([P, 512], F32R, tag="xs")
                    nc.sync.dma_start(
                        xs[:], xkv2_r[:, kt, (c4 - 2) * 512:(c4 - 1) * 512])
                    rhs = xs[:]
                nc.tensor.matmul(pl[:], wdown_sb[:, kt, :], rhs,
                                 start=(kt == 0), stop=(kt == DT - 1))
            nc.vector.tensor_tensor(
                latentT[:, c4 * 512:(c4 + 1) * 512], pl[:],
                bdown_sb[:].to_broadcast([P, 512]), ADD)

        # k projection: kT[128, DT, NKV] bf16 (head pair p8 on partitions)
        for p8 in range(DT):
            for c4 in range(4):
                pk = ps_a.tile([P, 512], F32, tag="acc")
                nc.tensor.matmul(pk[:], wkup_sb[:, p8 * P:(p8 + 1) * P],
                                 latentT[:, c4 * 512:(c4 + 1) * 512],
                                 start=True, stop=True)
                nc.vector.tensor_tensor(
                    kT[:, p8, c4 * 512:(c4 + 1) * 512], pk[:],
                    bk_sb[:, p8:p8 + 1].to_broadcast([P, 512]), ADD)

        # v projection: V_aug[128, NT, H, 65] bf16, col 64 = ones
        for rt in range(NT):
            for dc in range(2):
                pv = ps_a.tile([P, 512], F32, tag="acc")
                nc.tensor.matmul(pv[:], latentT[:, rt * P:(rt + 1) * P],
                                 wvup_sb[:, dc * 512:(dc + 1) * 512],
                                 start=True, stop=True)
                nc.vector.tensor_tensor(
                    v_aug[:, rt, dc * 8:(dc + 1) * 8, 0:HD],
                    pv[:].rearrange("p (h d) -> p h d", h=8),
                    bv_bc[:, dc * 512:(dc + 1) * 512].rearrange(
                        "p (h d) -> p h d", h=8), ADD)

        # q projection: qT[128, DT, NQ] bf16 (streams Wq column blocks)
        for t in range(DT):
            wq_col = ph1s.tile([P, DT, P], F32R, tag="wqc")
            nc.sync.dma_start(wq_col[:], wq_r[:, :, t * P:(t + 1) * P])
            for qc in range(QC):
                pq = ps_a.tile([P, 512], F32, tag="acc")
                for kt in range(DT):
                    nc.tensor.matmul(
                        pq[:], wq_col[:, kt, :],
                        xq_sb[:, kt, qc * 512:(qc + 1) * 512],
                        start=(kt == 0), stop=(kt == DT - 1))
                nc.vector.tensor_tensor(
                    qT[:, t, qc * 512:(qc + 1) * 512], pq[:],
                    bq_sb[:, t:t + 1].to_broadcast([P, 512]), ADD)

    # ---- attention ----
    att_ctx = ExitStack()
    epool = att_ctx.enter_context(tc.tile_pool(name="expst", bufs=36))
    small = att_ctx.enter_context(tc.tile_pool(name="small", bufs=4))
    for p8 in range(DT):
        for qc in range(QC):
            qs0 = qT[0:64, p8, qc * 512:(qc + 1) * 512]
            qs1 = qT[64:128, p8, qc * 512:(qc + 1) * 512]
            etiles = []
            for t in range(NT):
                s0 = ps_st.tile([P, 512], F32, tag="st")
                s1 = ps_st.tile([P, 512], F32, tag="st")
                nc.tensor.matmul(s0[:], kT[0:64, p8, t * P:(t + 1) * P], qs0,
                                 start=True, stop=True)
                nc.tensor.matmul(s1[:], kT[64:128, p8, t * P:(t + 1) * P], qs1,
                                 start=True, stop=True)
                e0 = epool.tile([P, 512], BF16, tag="e")
                e1 = epool.tile([P, 512], BF16, tag="e")
                nc.scalar.activation(e0[:], s0[:], EXP, scale=SCALE)
                nc.scalar.activation(e1[:], s1[:], EXP, scale=SCALE)
                etiles.append((e0, e1))
            pv0 = ps_pv.tile([P, 512], F32, tag="pv")
            pv1 = ps_pv.tile([P, 512], F32, tag="pv")
            for t in range(NT):
                nc.tensor.matmul(pv0[0:65, :], v_aug[:, t, 2 * p8, :],
                                 etiles[t][0][:],
                                 start=(t == 0), stop=(t == NT - 1))
                nc.tensor.matmul(pv1[0:65, :], v_aug[:, t, 2 * p8 + 1, :],
                                 etiles[t][1][:],
                                 start=(t == 0), stop=(t == NT - 1))
            # normalize: recip of denom row, broadcast to 64 partitions via PE
            for hh, pvh in ((0, pv0), (1, pv1)):
                rc = small.tile([65, 512], F32R, tag="rc")
                with nc.allow_low_precision(
                        reason="f32r recip feeds PE broadcast; ~2^-13 ok"):
                    nc.vector.reciprocal(rc[64:65, :], pvh[64:65, :])
                bcp = ps_bc.tile([128, 512], F32, tag="bc")
                nc.tensor.matmul(bcp[0:64, :], ones65[64:65, :], rc[64:65, :],
                                 start=True, stop=True)
                bcs = small.tile([64, 512], F32, tag="bcs")
                nc.vector.tensor_copy(bcs[:], bcp[0:64, :])
                if hh == 0:
                    nc.vector.tensor_tensor(
                        attnT[0:64, p8, qc * 512:(qc + 1) * 512],
                        pvh[0:64, :], bcs[:], MULT)
                else:
                    tmp = small.tile([64, 512], BF16, tag="tmp1")
                    nc.vector.tensor_tensor(tmp[:], pvh[0:64, :], bcs[:], MULT)
                    nc.sync.dma_start(
                        attnT[64:128, p8, qc * 512:(qc + 1) * 512], tmp[:])

    att_ctx.close()

    # ---- output projection ----
    with tc.tile_pool(name="ostage", bufs=3) as ostage:
        for qt in range(DT):
            for on in range(2):
                po = ps_a.tile([P, 512], F32, tag="acc")
                for kt in range(DT):
                    nc.tensor.matmul(po[:], attnT[:, kt, qt * P:(qt + 1) * P],
                                     wout_sb[:, kt, on * 512:(on + 1) * 512],
                                     start=(kt == 0), stop=(kt == DT - 1))
                ot = ostage.tile([P, 512], F32, tag="ot")
                nc.vector.tensor_tensor(
                    ot[:], po[:], bout_bc[:, on * 512:(on + 1) * 512], ADD)
                nc.sync.dma_start(
                    out[qt * P:(qt + 1) * P, on * 512:(on + 1) * 512], ot[:])


def build():
    nc = bacc.Bacc("TRN2", target_bir_lowering=False, debug=False)
    xq = nc.dram_tensor("xq", [D, NQ], F32R, kind="ExternalInput").ap()
    xkv2 = nc.dram_tensor("xkv2", [D, NKV - NQ], F32R, kind="ExternalInput").ap()
    wq = nc.dram_tensor("wq", [D, D], F32R, kind="ExternalInput").ap()
    wdown = nc.dram_tensor("wdown", [D, RK], F32R, kind="ExternalInput").ap()
    wkup = nc.dram_tensor("wkup", [RK, D], F32R, kind="ExternalInput").ap()
    wvup = nc.dram_tensor("wvup", [RK, D], F32R, kind="ExternalInput").ap()
    wout = nc.dram_tensor("wout", [D, D], BF16, kind="ExternalInput").ap()
    bq_t = nc.dram_tensor("bq", [D], F32, kind="ExternalInput").ap()
    bdown_t = nc.dram_tensor("bdown", [RK], F32, kind="ExternalInput").ap()
    bk_t = nc.dram_tensor("bk", [D], F32, kind="ExternalInput").ap()
    bv_t = nc.dram_tensor("bv", [D], F32, kind="ExternalInput").ap()
    bout_t = nc.dram_tensor("bout", [D], F32, kind="ExternalInput").ap()
    out = nc.dram_tensor("out", [NQ, D], F32, kind="ExternalOutput").ap()

    aps = (xq, xkv2, wq, wdown, wkup, wvup, wout,
           bq_t, bdown_t, bk_t, bv_t, bout_t, out)
    with tile.TileContext(nc) as tc, ExitStack() as ctx:
        _body(ctx, tc, aps)
    nc.compile()
    return nc


_NC = None


def _get_nc():
    global _NC
    if _NC is None:
        _NC = build()
    return _NC


def make_in_maps(x, Wq, bq, Wdown, bdown, Wk_up, bk, Wv_up, bv, Wout, bout):
    f32 = np.float32
    shared = {
        "wq": np.ascontiguousarray(Wq, dtype=f32),
        "wdown": np.ascontiguousarray(Wdown, dtype=f32),
        "wkup": np.ascontiguousarray(Wk_up, dtype=f32),
        "wvup": np.ascontiguousarray(Wv_up, dtype=f32),
        "wout": np.ascontiguousarray(Wout).astype(ml_dtypes.bfloat16),
        "bq": np.ascontiguousarray(bq, dtype=f32),
        "bdown": np.ascontiguousarray(bdown, dtype=f32),
        "bk": np.ascontiguousarray(bk, dtype=f32),
        "bv": np.ascontiguousarray(bv, dtype=f32),
        "bout": np.ascontiguousarray(bout, dtype=f32),
    }
    x = np.asarray(x, dtype=f32)
    in_maps = []
    for c in range(8):
        b, half = c // 2, c % 2
        mine = x[b, half * NQ:(half + 1) * NQ]      # (1024, 1024)
        other = x[b, (1 - half) * NQ:(2 - half) * NQ]
        m = dict(shared)
        m["xq"] = np.ascontiguousarray(mine.T)
        m["xkv2"] = np.ascontiguousarray(other.T)
        in_maps.append(m)
    return in_maps


def kernel(x, Wq, bq, Wdown, bdown, Wk_up, bk, Wv_up, bv, Wout, bout,
           **run_kwargs):
    nc = _get_nc()
    in_maps = make_in_maps(x, Wq, bq, Wdown, bdown, Wk_up, bk, Wv_up, bv,
                           Wout, bout)
    res = run_bass_kernel_spmd(nc, in_maps, core_ids=list(range(8)),
                               **run_kwargs)
    kernel.last_results = res
    out = np.empty((4, 2048, D), dtype=np.float32)
    for c in range(8):
        b, half = c // 2, c % 2
        out[b, half * NQ:(half + 1) * NQ] = res.results[c]["out"]
    return out


# revision 10
# speedup vs baseline: 124.0905x; 124.0905x over previous
"""Multi-head latent attention on 8 TRN2 NeuronCores (Bass/Tile).

Sharding: core c handles batch b=c//2, query rows half=c%2 (1024 rows).
Keys/values cover the full batch (2048 rows). The host passes x transposed
and "rolled" so each core's query rows are always columns 0-1023, keeping
the SPMD program uniform (softmax is permutation-invariant over keys).

On-chip layouts are transposed ([feature, row]). Scores are computed as
ST[k, q]; softmax denominators come for free from a ones-column appended
to V (M=65 PV matmul). Projections run in float32r (tf32-class precision
at full PE speed); attention tensors are bf16.
"""
import numpy as np
from contextlib import ExitStack

import ml_dtypes

import concourse.bass as bass
import concourse.tile as tile
from concourse import bacc, mybir
from concourse.bass_utils import run_bass_kernel_spmd

P = 128
D = 1024
NKV = 2048
NQ = 1024          # query rows per core
RK = 128           # kv latent rank
H = 16
HD = 64
SCALE = HD ** -0.5  # 0.125
NT = NKV // P      # 16 key-row tiles
DT = D // P        # 8 feature tiles
QC = NQ // 512     # 2 query chunks of 512

F32 = mybir.dt.float32
F32R = mybir.dt.float32r
BF16 = mybir.dt.bfloat16
ADD = mybir.AluOpType.add
MULT = mybir.AluOpType.mult
EXP = mybir.ActivationFunctionType.Exp


def _body(ctx: ExitStack, tc, aps):
    nc = tc.nc
    xq, xkv2, wq, wdown, wkup, wvup, wout, bq, bdown, bk, bv, bout, out = aps

    w1 = ctx.enter_context(tc.tile_pool(name="w1", bufs=1))
    big = ctx.enter_context(tc.tile_pool(name="big", bufs=1))
    ps_a = ctx.enter_context(tc.tile_pool(name="ps_a", bufs=2, space="PSUM"))
    ps_st = ctx.enter_context(tc.tile_pool(name="ps_st", bufs=3, space="PSUM"))
    ps_pv = ctx.enter_context(tc.tile_pool(name="ps_pv", bufs=2, space="PSUM"))
    ps_bc = ctx.enter_context(tc.tile_pool(name="ps_bc", bufs=1, space="PSUM"))

    # ---- weights / biases to SBUF ----
    wq_r = wq.rearrange("(k p) n -> p k n", p=P)
    wdown_sb = w1.tile([P, DT, RK], F32R)
    nc.sync.dma_start(wdown_sb[:], wdown.rearrange("(t p) r -> p t r", p=P))
    wkup_sb = w1.tile([P, D], F32R)
    nc.sync.dma_start(wkup_sb[:], wkup)
    wvup_sb = w1.tile([P, D], F32R)
    nc.sync.dma_start(wvup_sb[:], wvup)
    wout_sb = w1.tile([P, DT, D], BF16)
    nc.sync.dma_start(wout_sb[:], wout.rearrange("(t p) n -> p t n", p=P))

    bq_sb = w1.tile([P, DT], F32)
    nc.sync.dma_start(bq_sb[:], bq.rearrange("(t p) -> p t", p=P))
    bk_sb = w1.tile([P, DT], F32)
    nc.sync.dma_start(bk_sb[:], bk.rearrange("(t p) -> p t", p=P))
    bdown_sb = w1.tile([P, 1], F32)
    nc.sync.dma_start(bdown_sb[:], bdown.rearrange("(o p) -> p o", p=P))
    bv_row = w1.tile([1, D], F32)
    nc.sync.dma_start(bv_row[:], bv.rearrange("(o d) -> o d", o=1))
    bout_row = w1.tile([1, D], F32)
    nc.sync.dma_start(bout_row[:], bout.rearrange("(o d) -> o d", o=1))

    # broadcast bv/bout along partitions via K=1 matmul (fp32)
    ones1 = w1.tile([1, P], F32)
    nc.vector.memset(ones1[:], 1.0)
    bv_bc = w1.tile([P, D], F32)
    bout_bc = w1.tile([P, D], F32)
    for c in range(2):
        pb = ps_bc.tile([P, 512], F32, tag="bc")
        nc.tensor.matmul(pb[:], ones1[:], bv_row[:, c * 512:(c + 1) * 512],
                         start=True, stop=True)
        nc.vector.tensor_copy(bv_bc[:, c * 512:(c + 1) * 512], pb[:])
        pb2 = ps_bc.tile([P, 512], F32, tag="bc")
        nc.tensor.matmul(pb2[:], ones1[:], bout_row[:, c * 512:(c + 1) * 512],
                         start=True, stop=True)
        nc.vector.tensor_copy(bout_bc[:, c * 512:(c + 1) * 512], pb2[:])

    # ones for the denominator-broadcast matmul (row 64 = lane of the denom)
    ones65_f = w1.tile([65, HD], F32)
    nc.vector.memset(ones65_f[:], 1.0)
    ones65 = w1.tile([65, HD], F32R)
    nc.vector.tensor_copy(ones65[:], ones65_f[:])

    # persistent attention tensors
    qT = big.tile([P, DT, NQ], BF16)
    kT = big.tile([P, DT, NKV], BF16)
    v_aug = big.tile([P, NT, H, HD + 1], BF16)
    attnT = big.tile([P, DT, NQ], BF16)
    nc.vector.memset(v_aug[:], 1.0)

    # ---- phase 1: projections (scoped pools, released before attention) --
    with tc.tile_pool(name="ph1", bufs=1) as ph1, \
            tc.tile_pool(name="ph1s", bufs=4) as ph1s:
        xq_sb = ph1.tile([P, DT, NQ], F32R)
        nc.sync.dma_start(xq_sb[:], xq.rearrange("(t p) n -> p t n", p=P))
        xkv2_r = xkv2.rearrange("(t p) n -> p t n", p=P)
        latentT = ph1.tile([P, NKV], F32R)

        # latent projection: latentT[rank, all 2048 rows]
        for c4 in range(4):
            pl = ps_a.tile([P, 512], F32, tag="acc")
            for kt in range(DT):
                if c4 < 2:
                    rhs = xq_sb[:, kt, c4 * 512:(c4 + 1) * 512]
                else:
                    xs = ph1s.tile([P, 512], F32R, tag="xs")
                    nc.sync.dma_start(
                        xs[:], xkv2_r[:, kt, (c4 - 2) * 512:(c4 - 1) * 512])
                    rhs = xs[:]
                nc.tensor.matmul(pl[:], wdown_sb[:, kt, :], rhs,
                                 start=(kt == 0), stop=(kt == DT - 1))
            nc.vector.tensor_tensor(
                latentT[:, c4 * 512:(c4 + 1) * 512], pl[:],
                bdown_sb[:].to_broadcast([P, 512]), ADD)

        # k projection: kT[128, DT, NKV] bf16 (head pair p8 on partitions)
        for p8 in range(DT):
            for c4 in range(4):
                pk = ps_a.tile([P, 512], F32, tag="acc")
                nc.tensor.matmul(pk[:], wkup_sb[:, p8 * P:(p8 + 1) * P],
                                 latentT[:, c4 * 512:(c4 + 1) * 512],
                                 start=True, stop=True)
                nc.vector.tensor_tensor(
                    kT[:, p8, c4 * 512:(c4 + 1) * 512], pk[:],
                    bk_sb[:, p8:p8 + 1].to_broadcast([P, 512]), ADD)

        # v projection: V_aug[128, NT, H, 65] bf16, col 64 = ones
        for rt in range(NT):
            for dc in range(2):
                pv = ps_a.tile([P, 512], F32, tag="acc")
                nc.tensor.matmul(pv[:], latentT[:, rt * P:(rt + 1) * P],
                                 wvup_sb[:, dc * 512:(dc + 1) * 512],
                                 start=True, stop=True)
                nc.vector.tensor_tensor(
                    v_aug[:, rt, dc * 8:(dc + 1) * 8, 0:HD],
                    pv[:].rearrange("p (h d) -> p h d", h=8),
                    bv_bc[:, dc * 512:(dc + 1) * 512].rearrange(
                        "p (h d) -> p h d", h=8), ADD)

        # q projection: qT[128, DT, NQ] bf16 (streams Wq column blocks)
        for t in range(DT):
            wq_col = ph1s.tile([P, DT, P], F32R, tag="wqc")
            nc.sync.dma_start(wq_col[:], wq_r[:, :, t * P:(t + 1) * P])
            for qc in range(QC):
                pq = ps_a.tile([P, 512], F32, tag="acc")
                for kt in range(DT):
                    nc.tensor.matmul(
                        pq[:], wq_col[:, kt, :],
                        xq_sb[:, kt, qc * 512:(qc + 1) * 512],
                        start=(kt == 0), stop=(kt == DT - 1))
                nc.vector.tensor_tensor(
                    qT[:, t, qc * 512:(qc + 1) * 512], pq[:],
                    bq_sb[:, t:t + 1].to_broadcast([P, 512]), ADD)

    # ---- attention ----
    att_ctx = ExitStack()
    epool = att_ctx.enter_context(tc.tile_pool(name="expst", bufs=36))
    small = att_ctx.enter_context(tc.tile_pool(name="small", bufs=4))
    for p8 in range(DT):
        for qc in range(QC):
            qs0 = qT[0:64, p8, qc * 512:(qc + 1) * 512]
            qs1 = qT[64:128, p8, qc * 512:(qc + 1) * 512]
            etiles = []
            for t in range(NT):
                s0 = ps_st.tile([P, 512], F32, tag="st")
                s1 = ps_st.tile([P, 512], F32, tag="st")
                nc.tensor.matmul(s0[:], kT[0:64, p8, t * P:(t + 1) * P], qs0,
                                 start=True, stop=True)
                nc.tensor.matmul(s1[:], kT[64:128, p8, t * P:(t + 1) * P], qs1,
                                 start=True, stop=True)
                e0 = epool.tile([P, 512], BF16, tag="e")
                e1 = epool.tile([P, 512], BF16, tag="e")
                nc.scalar.activation(e0[:], s0[:], EXP, scale=SCALE)
                nc.scalar.activation(e1[:], s1[:], EXP, scale=SCALE)
                etiles.append((e0, e1))
            pv0 = ps_pv.tile([P, 512], F32, tag="pv")
            pv1 = ps_pv.tile([P, 512], F32, tag="pv")
            for t in range(NT):
                nc.tensor.matmul(pv0[0:65, :], v_aug[:, t, 2 * p8, :],
                                 etiles[t][0][:],
                                 start=(t == 0), stop=(t == NT - 1))
                nc.tensor.matmul(pv1[0:65, :], v_aug[:, t, 2 * p8 + 1, :],
                                 etiles[t][1][:],
                                 start=(t == 0), stop=(t == NT - 1))
            # normalize: recip of denom row, broadcast to 64 partitions via PE
            for hh, pvh in ((0, pv0), (1, pv1)):
                rc = small.tile([65, 512], F32R, tag="rc")
                with nc.allow_low_precision(
                        reason="f32r recip feeds PE broadcast; ~2^-13 ok"):
                    nc.vector.reciprocal(rc[64:65, :], pvh[64:65, :])
                bcp = ps_bc.tile([128, 512], F32, tag="bc")
                nc.tensor.matmul(bcp[0:64, :], ones65[64:65, :], rc[64:65, :],
                                 start=True, stop=True)
                bcs = small.tile([64, 512], F32, tag="bcs")
                nc.vector.tensor_copy(bcs[:], bcp[0:64, :])
                if hh == 0:
                    nc.vector.tensor_tensor(
                        attnT[0:64, p8, qc * 512:(qc + 1) * 512],
                        pvh[0:64, :], bcs[:], MULT)
                else:
                    tmp = small.tile([64, 512], BF16, tag="tmp1")
                    nc.vector.tensor_tensor(tmp[:], pvh[0:64, :], bcs[:], MULT)
                    nc.sync.dma_start(
                        attnT[64:128, p8, qc * 512:(qc + 1) * 512], tmp[:])

    att_ctx.close()

    # ---- output projection ----
    with tc.tile_pool(name="ostage", bufs=3) as ostage:
        for qt in range(DT):
            for on in range(2):
                po = ps_a.tile([P, 512], F32, tag="acc")
                for kt in range(DT):
                    nc.tensor.matmul(po[:], attnT[:, kt, qt * P:(qt + 1) * P],
                                     wout_sb[:, kt, on * 512:(on + 1) * 512],
                                     start=(kt == 0), stop=(kt == DT - 1))
                ot = ostage.tile([P, 512], F32, tag="ot")
                nc.vector.tensor_tensor(
                    ot[:], po[:], bout_bc[:, on * 512:(on + 1) * 512], ADD)
                nc.sync.dma_start(
                    out[qt * P:(qt + 1) * P, on * 512:(on + 1) * 512], ot[:])


def build(repeats=1):
    nc = bacc.Bacc("TRN2", target_bir_lowering=False, debug=False)
    xq = nc.dram_tensor("xq", [D, NQ], F32R, kind="ExternalInput").ap()
    xkv2 = nc.dram_tensor("xkv2", [D, NKV - NQ], F32R, kind="ExternalInput").ap()
    wq = nc.dram_tensor("wq", [D, D], F32R, kind="ExternalInput").ap()
    wdown = nc.dram_tensor("wdown", [D, RK], F32R, kind="ExternalInput").ap()
    wkup = nc.dram_tensor("wkup", [RK, D], F32R, kind="ExternalInput").ap()
    wvup = nc.dram_tensor("wvup", [RK, D], F32R, kind="ExternalInput").ap()
    wout = nc.dram_tensor("wout", [D, D], BF16, kind="ExternalInput").ap()
    bq_t = nc.dram_tensor("bq", [D], F32, kind="ExternalInput").ap()
    bdown_t = nc.dram_tensor("bdown", [RK], F32, kind="ExternalInput").ap()
    bk_t = nc.dram_tensor("bk", [D], F32, kind="ExternalInput").ap()
    bv_t = nc.dram_tensor("bv", [D], F32, kind="ExternalInput").ap()
    bout_t = nc.dram_tensor("bout", [D], F32, kind="ExternalInput").ap()
    out = nc.dram_tensor("out", [NQ, D], F32, kind="ExternalOutput").ap()

    aps = (xq, xkv2, wq, wdown, wkup, wvup, wout,
           bq_t, bdown_t, bk_t, bv_t, bout_t, out)
    with tile.TileContext(nc) as tc:
        for _ in range(repeats):
            with ExitStack() as ctx:
                _body(ctx, tc, aps)
    nc.compile()
    return nc


_NC = None


def _get_nc():
    global _NC
    if _NC is None:
        _NC = build()
    return _NC


def make_in_maps(x, Wq, bq, Wdown, bdown, Wk_up, bk, Wv_up, bv, Wout, bout):
    f32 = np.float32
    shared = {
        "wq": np.ascontiguousarray(Wq, dtype=f32),
        "wdown": np.ascontiguousarray(Wdown, dtype=f32),
        "wkup": np.ascontiguousarray(Wk_up, dtype=f32),
        "wvup": np.ascontiguousarray(Wv_up, dtype=f32),
        "wout": np.ascontiguousarray(Wout).astype(ml_dtypes.bfloat16),
        "bq": np.ascontiguousarray(bq, dtype=f32),
        "bdown": np.ascontiguousarray(bdown, dtype=f32),
        "bk": np.ascontiguousarray(bk, dtype=f32),
        "bv": np.ascontiguousarray(bv, dtype=f32),
        "bout": np.ascontiguousarray(bout, dtype=f32),
    }
    x = np.asarray(x, dtype=f32)
    in_maps = []
    for c in range(8):
        b, half = c // 2, c % 2
        mine = x[b, half * NQ:(half + 1) * NQ]      # (1024, 1024)
        other = x[b, (1 - half) * NQ:(2 - half) * NQ]
        m = dict(shared)
        m["xq"] = np.ascontiguousarray(mine.T)
        m["xkv2"] = np.ascontiguousarray(other.T)
        in_maps.append(m)
    return in_maps


def kernel(x, Wq, bq, Wdown, bdown, Wk_up, bk, Wv_up, bv, Wout, bout,
           **run_kwargs):
    nc = _get_nc()
    in_maps = make_in_maps(x, Wq, bq, Wdown, bdown, Wk_up, bk, Wv_up, bv,
                           Wout, bout)
    res = run_bass_kernel_spmd(nc, in_maps, core_ids=list(range(8)),
                               **run_kwargs)
    kernel.last_results = res
    out = np.empty((4, 2048, D), dtype=np.float32)
    for c in range(8):
        b, half = c // 2, c % 2
        out[b, half * NQ:(half + 1) * NQ] = res.results[c]["out"]
    return out
